# revision 1
# baseline (speedup 1.0000x reference)
"""Trainium2 Bass kernel for nn_MixtureOfExperts (B=524288, IN=59, E=4, H=64).

Strategy (pure data parallel over 8 cores, 65536 rows each):
 - Host folds BN into weights (scale into W, shift into per-feature bias),
   collapses expert head w3@wp -> wep (H->1), and pre-transposes x into a
   feature-on-partition layout so no on-chip transposes are needed.
 - On chip, everything is feature-major [feat, batch] with batch tiles of 512
   on the matmul moving dim.
 - Stage 1 + gating hidden run in float32r (full PE rate; ISA requires dst
   partition 0, so their outputs are full [128,*] tiles / zero-col-padded
   accumulations).  Stage 2 / preds / logits run in bf16 (dst partition can
   be 32-aligned, enabling strip packing + 4-way quadrant concurrency).
 - x is packed [128, S] with two independent 64-feature batch-halves on
   partition halves, so stage-1 matmuls for consecutive tiles land on
   disjoint PE row strips and overlap on the array.
 - Tiny outputs (gate hidden, logits, per-expert preds) are packed into full
   [128, 512] PSUM tiles via strips / zero-padded lhsT columns so the
   PSUM->SBUF hops always run with all 128 lanes busy.
 - softmax-weighted combine: pred = sum_e exp(l_e)*(p_e+b_e) / sum_e exp(l_e)
   (no max-subtraction needed; logits are O(1)).
"""

import numpy as np
import ml_dtypes

import concourse.bass as bass
import concourse.mybir as mybir
import concourse.tile as tile
from concourse import bacc
from concourse.bass_utils import run_bass_kernel_spmd

F32 = mybir.dt.float32
F32R = mybir.dt.float32r
BF16 = mybir.dt.bfloat16
AF = mybir.ActivationFunctionType
ALU = mybir.AluOpType

B, IN, E, H, EMB, GH = 524288, 59, 4, 64, 32, 32
EPS = 1e-5
NCORES = 8
BC = B // NCORES          # 65536 rows per core
S = 8192                  # rows per batch-half per superstep
SUP = BC // (2 * S)       # 4 supersteps
NT = (2 * S) // 512       # 32 tiles per superstep
BT = 512
W_F = 768 + 32 + 8        # f32r wts width: w1(256)+gate(512) | gsum | bias
W_B = 128 + 512 + 1024    # bf16 wts width: w2 | wep | gw2

_CACHE = {}


def _build():
    nc = bacc.Bacc(trn_type="TRN2")
    x_d = nc.dram_tensor("x", (SUP, 128, S), F32R, kind="ExternalInput")
    wts_d = nc.dram_tensor("wts", (128, W_F), F32R, kind="ExternalInput")
    wtsb_d = nc.dram_tensor("wtsb", (128, W_B), BF16, kind="ExternalInput")
    out_d = nc.dram_tensor("out", (SUP, NT, BT), F32, kind="ExternalOutput")

    with tile.TileContext(nc) as tc:
        with (
            tc.tile_pool(name="consts", bufs=1) as consts,
            tc.tile_pool(name="xp", bufs=2) as xp,
            tc.tile_pool(name="hs", bufs=2) as hs,
            tc.tile_pool(name="gts", bufs=2) as gts,
            tc.tile_pool(name="tails", bufs=2) as tails,
            tc.tile_pool(name="ph1a", bufs=1, space="PSUM") as ph1a,
            tc.tile_pool(name="ph1b", bufs=1, space="PSUM") as ph1b,
            tc.tile_pool(name="ph2a", bufs=1, space="PSUM") as ph2a,
            tc.tile_pool(name="ph2b", bufs=1, space="PSUM") as ph2b,
            tc.tile_pool(name="pga", bufs=1, space="PSUM") as pga,
            tc.tile_pool(name="pgb", bufs=1, space="PSUM") as pgb,
            tc.tile_pool(name="pl", bufs=1, space="PSUM") as pl,
            tc.tile_pool(name="pp", bufs=1, space="PSUM") as pp,
        ):
            wts_sb = consts.tile([128, W_F], F32R)
            nc.sync.dma_start(out=wts_sb, in_=wts_d[:, :])
            wtsb_sb = consts.tile([128, W_B], BF16)
            nc.sync.dma_start(out=wtsb_sb, in_=wtsb_d[:, :])
            w1_sb = wts_sb[:, 0:768]          # 0:256 experts, 256:768 gate(4x128)
            gs_sb = wts_sb[:, 768:800]
            bias_sb = wts_sb[:, 800:808].bitcast(F32)
            w2_sb = wtsb_sb[:, 0:128]
            wep_sb = wtsb_sb[:, 128:640].rearrange(
                "p (h j m) -> p h j m", h=2, j=8)
            gw2_sb = wtsb_sb[:, 640:1664].rearrange(
                "p (g j m) -> p g j m", g=4, j=8)
            c2a = bias_sb[:, 0:1]
            c2b = bias_sb[:, 1:2]
            gb2t = bias_sb[:, 2:3]
            bept = bias_sb[:, 3:4]

            for k in range(SUP):
                x_sb = xp.tile([128, S], F32R, tag="x")
                for ch in range(4):
                    cw = S // 4
                    nc.sync.dma_start(
                        out=x_sb[:, ch * cw : (ch + 1) * cw],
                        in_=x_d[k][:, ch * cw : (ch + 1) * cw])

                l_ps = pl.tile([128, BT], F32, tag="l")
                p_ps = pp.tile([128, BT], F32, tag="p")

                for q in range(4):            # group = pairs 4q..4q+3
                    # ---- gating: per-half accumulators (an f32r matmul
                    # group must keep one row base; mixing 0/64 into the
                    # same PSUM tile crashes the device).  4 zero-col-padded
                    # M=128 lhsT slots pack 4 tiles per [128,512] bank.
                    ga_ps = pga.tile([128, BT], F32, tag="ga")
                    gb_ps = pgb.tile([128, BT], F32, tag="gb")
                    for gi in range(4):
                        cg = (4 * q + gi) * BT
                        lt = w1_sb[:, 256 + 128 * gi : 384 + 128 * gi]
                        nc.tensor.matmul(
                            out=ga_ps,
                            lhsT=lt[0:64, :],
                            rhs=x_sb[0:64, cg : cg + BT],
                            start=(gi == 0), stop=(gi == 3),
                            skip_group_check=True,
                        )
                        nc.tensor.matmul(
                            out=gb_ps,
                            lhsT=lt[64:128, :],
                            rhs=x_sb[64:128, cg : cg + BT],
                            start=(gi == 0), stop=(gi == 3),
                            skip_group_check=True,
                        )
                    g1a_sb = gts.tile([128, BT], BF16, tag="g1a")
                    nc.scalar.activation(g1a_sb, ga_ps, AF.Relu)
                    g1b_sb = gts.tile([128, BT], BF16, tag="g1b")
                    nc.scalar.activation(g1b_sb, gb_ps, AF.Relu)

                    for pi in range(4):       # pair inside group
                        pr = 4 * q + pi
                        c0 = pr * BT
                        for half in (0, 1):
                            t = pr + 16 * half
                            base = 64 * half
                            strip = t // 8
                            j = t % 8
                            gslot = pi
                            g1_sb = g1a_sb if half == 0 else g1b_sb
                            xs = x_sb[base : base + 64, c0 : c0 + BT]

                            # ---- stage 1 (f32r, dst 0, M=128).  Biases are
                            # folded into the matmul via the ones-row of x
                            # (row 59 carries c1/gb1 in the weights).
                            h1a_ps = ph1a.tile([128, BT], F32, tag="h1a")
                            nc.tensor.matmul(
                                out=h1a_ps,
                                lhsT=w1_sb[base : base + 64, 0:128],
                                rhs=xs, start=True, stop=True,
                            )
                            h1b_ps = ph1b.tile([128, BT], F32, tag="h1b")
                            nc.tensor.matmul(
                                out=h1b_ps,
                                lhsT=w1_sb[base : base + 64, 128:256],
                                rhs=xs, start=True, stop=True,
                            )
                            h1a_sb = hs.tile([128, BT], BF16, tag="h1as")
                            nc.scalar.activation(h1a_sb, h1a_ps, AF.Relu)
                            h1b_sb = hs.tile([128, BT], BF16, tag="h1bs")
                            nc.vector.tensor_scalar(
                                h1b_sb, h1b_ps, 0.0, None, ALU.max)

                            # ---- stage 2 (bf16): 4 concurrent quadrants
                            h2a_ps = ph2a.tile([128, BT], F32, tag="h2a")
                            h2b_ps = ph2b.tile([128, BT], F32, tag="h2b")
                            nc.tensor.matmul(   # e0
                                out=h2a_ps[0:64, :], lhsT=w2_sb[0:64, 0:64],
                                rhs=h1a_sb[0:64, :], start=True, stop=True)
                            nc.tensor.matmul(   # e1
                                out=h2a_ps[64:128, :],
                                lhsT=w2_sb[64:128, 0:64],
                                rhs=h1a_sb[64:128, :], start=True, stop=True)
                            nc.tensor.matmul(   # e2 -> h2b[64:]
                                out=h2b_ps[64:128, :],
                                lhsT=w2_sb[0:64, 64:128],
                                rhs=h1b_sb[0:64, :], start=True, stop=True)
                            nc.tensor.matmul(   # e3 -> h2b[:64]
                                out=h2b_ps[0:64, :],
                                lhsT=w2_sb[64:128, 64:128],
                                rhs=h1b_sb[64:128, :], start=True, stop=True)
                            h2a_sb = hs.tile([128, BT], BF16, tag="h2as")
                            nc.scalar.activation(
                                h2a_sb, h2a_ps, AF.Relu, bias=c2a)
                            h2b_sb = hs.tile([128, BT], BF16, tag="h2bs")
                            nc.vector.tensor_scalar(
                                h2b_sb, h2b_ps, c2b, 0.0, ALU.add, ALU.max)

                            # ---- stage 3 (bf16): preds into p_ps strip
                            nc.tensor.matmul(
                                out=p_ps[32 * strip : 32 * strip + 32, :],
                                lhsT=wep_sb[:, 0, j, :],
                                rhs=h2a_sb,
                                start=(j == 0), stop=False,
                                skip_group_check=True,
                                tile_position=(0, 32 * strip),
                            )
                            nc.tensor.matmul(
                                out=p_ps[32 * strip : 32 * strip + 32, :],
                                lhsT=wep_sb[:, 1, j, :],
                                rhs=h2b_sb,
                                start=False, stop=(j == 7),
                                skip_group_check=True,
                                tile_position=(0, 32 * strip),
                            )

                            # ---- logits (bf16) into l_ps strip.  K=128
                            # with zero rows outside this gslot's strip so
                            # every mm in the accumulation group keeps row
                            # base 0 (mixed row bases crash the device).
                            nc.tensor.matmul(
                                out=l_ps[32 * strip : 32 * strip + 32, :],
                                lhsT=gw2_sb[:, gslot, j, :],
                                rhs=g1_sb,
                                start=(j == 0), stop=(j == 7),
                                skip_group_check=True,
                                tile_position=(0, 32 * strip),
                            )

                # ---- superstep tail (16384 rows), all full-width ops
                expl_sb = tails.tile([128, BT], F32R, tag="expl")
                nc.scalar.activation(expl_sb, l_ps, AF.Exp, bias=gb2t)
                pb_sb = tails.tile([128, BT], F32R, tag="pb")
                nc.vector.tensor_scalar(pb_sb, p_ps, bept, None, ALU.add)
                w_sb = tails.tile([128, BT], F32R, tag="wsb")
                nc.vector.tensor_mul(w_sb, pb_sb, expl_sb)

                num_ps = pl.tile([32, BT], F32, tag="l")
                nc.tensor.matmul(
                    out=num_ps, lhsT=gs_sb, rhs=w_sb, start=True, stop=True)
                den_ps = pp.tile([32, BT], F32, tag="p")
                nc.tensor.matmul(
                    out=den_ps, lhsT=gs_sb, rhs=expl_sb, start=True,
                    stop=True)
                denr_sb = tails.tile([32, BT], F32, tag="denr")
                out_sb = tails.tile([32, BT], F32, tag="outs")
                nc.vector.reciprocal(denr_sb, den_ps)
                nc.vector.tensor_mul(out_sb, num_ps, denr_sb)
                nc.sync.dma_start(out=out_d[k], in_=out_sb)

    if not nc.is_finalized():
        nc.finalize()
    return nc


def _pack_host(w1, b1, bn1_g, bn1_b, bn1_m, bn1_v, w2, b2, bn2_g, bn2_b,
               bn2_m, bn2_v, w3, b3, wp, bp, gw1, gb1, gw2, gb2):
    f = np.float32
    s1 = (bn1_g / np.sqrt(bn1_v + EPS)).astype(f)              # (E,H)
    w1e = (w1 * s1[:, None, :]).astype(f)                       # (E,IN,H)
    c1 = ((b1 - bn1_m) * s1 + bn1_b).astype(f)                  # (E,H)
    s2 = (bn2_g / np.sqrt(bn2_v + EPS)).astype(f)
    w2e = (w2 * s2[:, None, :]).astype(f)                       # (E,H,H)
    c2 = ((b2 - bn2_m) * s2 + bn2_b).astype(f)                  # (E,H)
    wep = np.einsum("ehm,em->eh", w3, wp).astype(f)             # (E,H)
    bep = (np.einsum("em,em->e", b3, wp) + bp).astype(f)        # (E,)

    # ---- f32r block: w1 experts (256) + gate 4 slots (4x128) + gsum + bias
    w1p = np.zeros((128, 768), f)
    half = np.zeros((64, 768), f)
    half[:IN, 0:64] = w1e[0]
    half[:IN, 64:128] = w1e[1]
    half[:IN, 128:192] = w1e[2]
    half[:IN, 192:256] = w1e[3]
    half[IN, 0:256] = np.concatenate([c1[0], c1[1], c1[2], c1[3]])
    for gi in range(4):
        lo = 256 + 128 * gi + 32 * gi
        half[:IN, lo : lo + 32] = gw1
        half[IN, lo : lo + 32] = gb1
    w1p[0:64] = half
    w1p[64:128] = half

    gsump = np.zeros((128, 32), f)
    for p in range(128):
        gsump[p, 8 * (p // 32) + (p % 32) // 4] = 1.0

    biasp = np.zeros((128, 8), f)
    biasp[:, 0] = np.concatenate([c2[0], c2[1]])
    biasp[:, 1] = np.concatenate([c2[3], c2[2]])   # h2b = [e3; e2]
    biasp[:, 2] = np.tile(gb2, 32)
    biasp[:, 3] = np.tile(bep, 32)

    wts = np.concatenate([w1p, gsump, biasp], axis=1)
    assert wts.shape == (128, W_F), wts.shape

    # ---- bf16 block: w2 quadrants + wep slots + gw2 slots
    w2p = np.zeros((128, 128), f)
    w2p[0:64, 0:64] = w2e[0]
    w2p[64:128, 0:64] = w2e[1]
    w2p[0:64, 64:128] = w2e[2]
    w2p[64:128, 64:128] = w2e[3]

    wepp = np.zeros((128, 2, 8, 32), f)
    for j in range(8):
        wepp[0:64, 0, j, 4 * j + 0] = wep[0]
        wepp[64:128, 0, j, 4 * j + 1] = wep[1]
        wepp[64:128, 1, j, 4 * j + 2] = wep[2]   # h2b = [e3; e2]
        wepp[0:64, 1, j, 4 * j + 3] = wep[3]

    gw2p = np.zeros((128, 4, 8, 32), f)
    for g in range(4):
        for j in range(8):
            gw2p[32 * g : 32 * g + 32, g, j, 4 * j : 4 * j + 4] = gw2

    wtsb = np.concatenate(
        [w2p, wepp.reshape(128, 512), gw2p.reshape(128, 1024)], axis=1)
    assert wtsb.shape == (128, W_B), wtsb.shape
    return dict(wts=np.ascontiguousarray(wts),
                wtsb=np.ascontiguousarray(wtsb.astype(ml_dtypes.bfloat16)))


def kernel(**inputs):
    x = np.asarray(inputs["x"], dtype=np.float32)
    wk = {k: np.asarray(v, dtype=np.float32) for k, v in inputs.items()
          if k != "x"}
    packed = _pack_host(**wk)

    if "nc" not in _CACHE:
        _CACHE["nc"] = _build()
    nc = _CACHE["nc"]

    in_maps = []
    for c in range(NCORES):
        xc = x[c * BC : (c + 1) * BC]                 # (BC, 59)
        xt = np.zeros((64, BC), np.float32)
        xt[:IN] = xc.T
        xt[IN] = 1.0
        xi = np.ascontiguousarray(
            xt.reshape(64, SUP, 2, S).transpose(1, 2, 0, 3).reshape(SUP, 128, S)
        )
        m = {"x": xi}
        m.update(packed)
        in_maps.append(m)

    res = run_bass_kernel_spmd(nc, in_maps, core_ids=list(range(NCORES)))
    _CACHE["last"] = res
    outs = [r["out"].reshape(BC) for r in res.results]
    return np.concatenate(outs).reshape(B, 1).astype(np.float32)



# revision 7
# speedup vs baseline: 1.4950x; 1.4950x over previous
"""Trainium2 Bass kernel for nn_MixtureOfExperts (B=524288, IN=59, E=4, H=64).

Strategy (pure data parallel over 8 cores, 65536 rows each):
 - Host folds BN into weights, collapses the expert head w3@wp -> wep, folds
   |wep| into w2's columns (so stage-3 reduction weights become exact +-1
   signs), and pre-transposes x into feature-major [64, BC] bf16 with a ones
   row at feature 59 (used to inject biases via accumulating matmuls).
 - Everything on-chip is bf16 matmul + f32 psum.  Per 512-row tile:
     stage1: 2 weights-stationary matmuls (experts 01 / 23), N=512
     gate hidden: 1 matmul into a 32-partition strip of a shared psum
     stage2: 2 block-diagonal K=128 matmuls, N=512
     stage3 preds / gate logits / biases: DATA-stationary matmuls - the
       activations (feature-major, in SBUF after relu) are the stationary
       operand and the tiny reduction weights stream, so each costs only
       N=2..8 moving columns.  Outputs land batch-major in one shared psum
       "tail" bank per 8192 rows: chunk cc -> cols [8cc:8cc+8] hold
       [p0 p1 p2 p3 l0 l1 l2 l3] for 128 rows.
 - Relu passes (psum->SBUF bf16 with per-partition bias) are spread across
   the Act, DVE and GPSIMD engines.
 - Tail per 8192 rows: spill bank to SBUF, exp(logits), w = p*exp, row
   reductions over the 4 experts, reciprocal, final product, DMA out.
"""

import numpy as np
import ml_dtypes

import concourse.bass as bass
import concourse.mybir as mybir
import concourse.tile as tile
from concourse import bacc
from concourse.bass_utils import run_bass_kernel_spmd

F32 = mybir.dt.float32
BF16 = mybir.dt.bfloat16
AF = mybir.ActivationFunctionType
ALU = mybir.AluOpType
AX = mybir.AxisListType

B, IN, E, H, EMB, GH = 524288, 59, 4, 64, 32, 32
EPS = 1e-5
NCORES = 8
BC = B // NCORES            # 65536 rows per core
NR = 8                      # rounds per core
RS = BC // NR               # 8192 rows per round
NT = RS // 512              # 16 tiles of 512 per round

# wb (bf16) column layout
W1A0, W1B0, GW10, W2A0, W2B0 = 0, 128, 256, 288, 416
SGA0, SGB0, GW2R0, BEP0 = 544, 546, 548, 552
WB_W = 560
# wf (f32) column layout: c1a c1b c2a c2b gb1t
WF_W = 8

_CACHE = {}

# relu engine assignment: per 16-tile round there are 68 psum->SBUF relu
# passes (64 tile + 4 gate).  GPSIMD cannot touch PSUM (BIR verifier), so
# they split across Act/DVE; Act is slightly faster per pass but also runs
# the exp, DVE runs the reductions/reciprocal.
def _relu_engines(total=68, quota=None):
    quota = quota or {"act": 36, "dve": 32}
    order = []
    frac = {k: 0.0 for k in quota}
    for _ in range(total):
        for k in frac:
            frac[k] += quota[k] / total
        pick = max(frac, key=lambda k: frac[k])
        frac[pick] -= 1.0
        order.append(pick)
    return order

RELU_ENG = _relu_engines()


def _build():
    nc = bacc.Bacc(trn_type="TRN2")
    x_d = nc.dram_tensor("x", (64, BC), BF16, kind="ExternalInput")
    wb_d = nc.dram_tensor("wb", (128, WB_W), BF16, kind="ExternalInput")
    wf_d = nc.dram_tensor("wf", (128, WF_W), F32, kind="ExternalInput")
    out_d = nc.dram_tensor("out", (NR, 128, 64), F32, kind="ExternalOutput")

    with tile.TileContext(nc) as tc:
        with (
            tc.tile_pool(name="consts", bufs=1) as consts,
            tc.tile_pool(name="xp", bufs=2) as xp,
            tc.tile_pool(name="h1p", bufs=2) as h1p,
            tc.tile_pool(name="h2p", bufs=2) as h2p,
            tc.tile_pool(name="g1p", bufs=2) as g1p,
            tc.tile_pool(name="tp", bufs=2) as tp,
            tc.tile_pool(name="p1a", bufs=2, space="PSUM") as p1ap,
            tc.tile_pool(name="p1b", bufs=2, space="PSUM") as p1bp,
            tc.tile_pool(name="p2a", bufs=1, space="PSUM") as p2ap,
            tc.tile_pool(name="p2b", bufs=1, space="PSUM") as p2bp,
            tc.tile_pool(name="pga", bufs=1, space="PSUM") as pgap,
            tc.tile_pool(name="ptl", bufs=1, space="PSUM") as ptlp,
        ):
            wb = consts.tile([128, WB_W], BF16)
            nc.sync.dma_start(out=wb, in_=wb_d[:, :])
            wf = consts.tile([128, WF_W], F32)
            nc.sync.dma_start(out=wf, in_=wf_d[:, :])

            w1a = wb[0:64, W1A0:W1A0 + 128]
            w1b = wb[0:64, W1B0:W1B0 + 128]
            gw1 = wb[0:64, GW10:GW10 + 32]
            w2a = wb[:, W2A0:W2A0 + 128]
            w2b = wb[:, W2B0:W2B0 + 128]
            sga = wb[:, SGA0:SGA0 + 2]
            sgb = wb[:, SGB0:SGB0 + 2]
            gw2r = wb[:, GW2R0:GW2R0 + 4]
            bep8 = wb[0:64, BEP0:BEP0 + 8]
            c1a = wf[:, 0:1]
            c1b = wf[:, 1:2]
            c2a = wf[:, 2:3]
            c2b = wf[:, 3:4]
            gb1t = wf[:, 4:5]

            def relu(eng, out_sb, in_ps, bias_ap):
                if eng == "act":
                    nc.scalar.activation(out_sb, in_ps, AF.Relu, bias=bias_ap)
                elif eng == "dve":
                    nc.vector.tensor_scalar(
                        out_sb, in_ps, bias_ap, 0.0, ALU.add, ALU.max)
                else:
                    nc.gpsimd.tensor_scalar(
                        out_sb, in_ps, bias_ap, 0.0, ALU.add, ALU.max)

            for r in range(NR):
                x_sb = xp.tile([64, RS], BF16, tag="x")
                for ch in range(2):
                    cw = RS // 2
                    nc.sync.dma_start(
                        out=x_sb[:, ch * cw:(ch + 1) * cw],
                        in_=x_d[:, r * RS + ch * cw: r * RS + (ch + 1) * cw])

                tail = ptlp.tile([128, 512], F32, tag="tail")
                ri = 0  # relu slot index within round

                for g in range(4):
                    ga = pgap.tile([128, 512], F32, tag="ga")
                    g1r = g1p.tile([128, 512], BF16, tag="g1r")
                    h2s = []
                    for i in range(4):
                        t = 4 * g + i
                        c0 = 512 * t
                        xs = x_sb[:, c0:c0 + 512]

                        p1a = p1ap.tile([128, 512], F32, tag="p1a")
                        nc.tensor.matmul(
                            out=p1a, lhsT=w1a, rhs=xs, start=True, stop=True)
                        p1b = p1bp.tile([128, 512], F32, tag="p1b")
                        nc.tensor.matmul(
                            out=p1b, lhsT=w1b, rhs=xs, start=True, stop=True)
                        # gate hidden strip for this tile
                        nc.tensor.matmul(
                            out=ga[32 * i:32 * i + 32, :], lhsT=gw1, rhs=xs,
                            start=True, stop=True, skip_group_check=True,
                            tile_position=(0, 32 * i))

                        h1ra = h1p.tile([128, 512], BF16, tag="h1ra")
                        relu(RELU_ENG[ri], h1ra, p1a, c1a); ri += 1
                        h1rb = h1p.tile([128, 512], BF16, tag="h1rb")
                        relu(RELU_ENG[ri], h1rb, p1b, c1b); ri += 1

                        p2a = p2ap.tile([128, 512], F32, tag="p2a")
                        nc.tensor.matmul(
                            out=p2a, lhsT=w2a, rhs=h1ra, start=True, stop=True)
                        p2b = p2bp.tile([128, 512], F32, tag="p2b")
                        nc.tensor.matmul(
                            out=p2b, lhsT=w2b, rhs=h1rb, start=True, stop=True)

                        h2ra = h2p.tile([128, 512], BF16, tag="h2ra")
                        relu(RELU_ENG[ri], h2ra, p2a, c2a); ri += 1
                        h2rb = h2p.tile([128, 512], BF16, tag="h2rb")
                        relu(RELU_ENG[ri], h2rb, p2b, c2b); ri += 1
                        h2s.append((t, c0, h2ra, h2rb))

                        for q in range(4):
                            col = 8 * (4 * t + q)
                            # bias row first: writes all 8 cols fresh with
                            # (bep, gb2) via the x ones-row; preds/logits
                            # then accumulate on top.
                            nc.tensor.matmul(
                                out=tail[:, col:col + 8],
                                lhsT=x_sb[:, c0 + 128 * q:c0 + 128 * q + 128],
                                rhs=bep8,
                                start=(t == 0 and q == 0), stop=False,
                                skip_group_check=True)
                            nc.tensor.matmul(
                                out=tail[:, col:col + 2],
                                lhsT=h2ra[:, 128 * q:128 * q + 128],
                                rhs=sga,
                                start=False, stop=False,
                                skip_group_check=True)
                            nc.tensor.matmul(
                                out=tail[:, col + 2:col + 4],
                                lhsT=h2rb[:, 128 * q:128 * q + 128],
                                rhs=sgb,
                                start=False, stop=False,
                                skip_group_check=True)

                    relu(RELU_ENG[ri], g1r, ga, gb1t); ri += 1

                    for (t, c0, _, _) in h2s:
                        i = t - 4 * g
                        for q in range(4):
                            col = 8 * (4 * t + q) + 4
                            nc.tensor.matmul(
                                out=tail[:, col:col + 4],
                                lhsT=g1r[32 * i:32 * i + 32,
                                         128 * q:128 * q + 128],
                                rhs=gw2r[32 * i:32 * i + 32, :],
                                start=False, stop=(g == 3 and q == 3
                                                   and t == 4 * g + 3),
                                skip_group_check=True,
                                tile_position=(32 * i, 0))

                # ---- round tail: spill and combine (batch-major)
                tacc = tp.tile([128, 512], F32, tag="tacc")
                nc.scalar.activation(tacc, tail, AF.Copy)
                tv = tacc.rearrange("p (c k) -> p c k", k=8)
                expl = tp.tile([128, 256], F32, tag="expl")
                nc.scalar.activation(
                    expl.rearrange("p (c k) -> p c k", k=4),
                    tv[:, :, 4:8], AF.Exp)
                w_sb = tp.tile([128, 256], F32, tag="w")
                nc.gpsimd.tensor_mul(
                    w_sb.rearrange("p (c k) -> p c k", k=4),
                    tv[:, :, 0:4],
                    expl.rearrange("p (c k) -> p c k", k=4))
                num = tp.tile([128, 64], F32, tag="num")
                nc.vector.tensor_reduce(
                    num, w_sb.rearrange("p (c k) -> p c k", k=4),
                    AX.X, ALU.add)
                den = tp.tile([128, 64], F32, tag="den")
                nc.vector.tensor_reduce(
                    den, expl.rearrange("p (c k) -> p c k", k=4),
                    AX.X, ALU.add)
                rec = tp.tile([128, 64], F32, tag="rec")
                nc.vector.reciprocal(rec, den)
                o_sb = tp.tile([128, 64], F32, tag="o")
                nc.gpsimd.tensor_mul(o_sb, num, rec)
                nc.sync.dma_start(out=out_d[r], in_=o_sb)

    if not nc.is_finalized():
        nc.finalize()
    return nc


def _pack_host(w1, b1, bn1_g, bn1_b, bn1_m, bn1_v, w2, b2, bn2_g, bn2_b,
               bn2_m, bn2_v, w3, b3, wp, bp, gw1, gb1, gw2, gb2):
    f = np.float32
    s1 = (bn1_g / np.sqrt(bn1_v + EPS)).astype(f)
    w1e = (w1 * s1[:, None, :]).astype(f)                       # (E,IN,H)
    c1 = ((b1 - bn1_m) * s1 + bn1_b).astype(f)                  # (E,H)
    s2 = (bn2_g / np.sqrt(bn2_v + EPS)).astype(f)
    w2e = (w2 * s2[:, None, :]).astype(f)                       # (E,H,H)
    c2 = ((b2 - bn2_m) * s2 + bn2_b).astype(f)                  # (E,H)
    wep = np.einsum("ehm,em->eh", w3, wp).astype(f)             # (E,H)
    bep = (np.einsum("em,em->e", b3, wp) + bp).astype(f)        # (E,)

    aw = np.abs(wep)                                            # (E,H)
    sg = np.sign(wep).astype(f)
    w2f = w2e * aw[:, None, :]                                  # cols scaled
    c2f = c2 * aw

    wb = np.zeros((128, WB_W), f)
    wb[0:IN, W1A0:W1A0 + 64] = w1e[0]
    wb[0:IN, W1A0 + 64:W1A0 + 128] = w1e[1]
    wb[0:IN, W1B0:W1B0 + 64] = w1e[2]
    wb[0:IN, W1B0 + 64:W1B0 + 128] = w1e[3]
    wb[0:IN, GW10:GW10 + 32] = gw1
    wb[0:64, W2A0:W2A0 + 64] = w2f[0]
    wb[64:128, W2A0 + 64:W2A0 + 128] = w2f[1]
    wb[0:64, W2B0:W2B0 + 64] = w2f[2]
    wb[64:128, W2B0 + 64:W2B0 + 128] = w2f[3]
    wb[0:64, SGA0] = sg[0]
    wb[64:128, SGA0 + 1] = sg[1]
    wb[0:64, SGB0] = sg[2]
    wb[64:128, SGB0 + 1] = sg[3]
    for gi in range(4):
        wb[32 * gi:32 * gi + 32, GW2R0:GW2R0 + 4] = gw2
    wb[IN, BEP0:BEP0 + 4] = bep
    wb[IN, BEP0 + 4:BEP0 + 8] = gb2

    wf = np.zeros((128, WF_W), f)
    wf[:, 0] = np.concatenate([c1[0], c1[1]])
    wf[:, 1] = np.concatenate([c1[2], c1[3]])
    wf[:, 2] = np.concatenate([c2f[0], c2f[1]])
    wf[:, 3] = np.concatenate([c2f[2], c2f[3]])
    wf[:, 4] = np.tile(gb1, 4)
    return dict(wb=wb.astype(ml_dtypes.bfloat16), wf=wf)


def _x_core(xc):
    """(BC, 59) f32 -> (64, BC) bf16 feature-major with ones row at 59."""
    xt = np.zeros((64, BC), np.float32)
    xt[:IN] = xc.T
    xt[IN] = 1.0
    return np.ascontiguousarray(xt).astype(ml_dtypes.bfloat16)


def _unpack_out(o):
    """(NR, 128, 64) -> (BC,): row = 8192 r + 512 t + 128 q + b, col=4t+q."""
    o = np.asarray(o, np.float32).reshape(NR, 128, NT, 4)
    return np.ascontiguousarray(o.transpose(0, 2, 3, 1)).reshape(BC)


def _sim_inputs(x_full, packed):
    m = {"x": _x_core(np.asarray(x_full, np.float32)[:BC])}
    m.update(packed)
    return m


def kernel(**inputs):
    x = np.asarray(inputs["x"], dtype=np.float32)
    wk = {k: np.asarray(v, dtype=np.float32) for k, v in inputs.items()
          if k != "x"}
    packed = _pack_host(**wk)

    if "nc" not in _CACHE:
        _CACHE["nc"] = _build()
    nc = _CACHE["nc"]

    in_maps = []
    for c in range(NCORES):
        m = {"x": _x_core(x[c * BC:(c + 1) * BC])}
        m.update(packed)
        in_maps.append(m)

    res = run_bass_kernel_spmd(nc, in_maps, core_ids=list(range(NCORES)))
    _CACHE["last"] = res
    outs = [_unpack_out(r["out"]) for r in res.results]
    return np.concatenate(outs).reshape(B, 1).astype(np.float32)


# revision 10
# speedup vs baseline: 1.5405x; 1.0304x over previous
"""Trainium2 Bass kernel for nn_MixtureOfExperts (B=524288, IN=59, E=4, H=64).

Strategy (pure data parallel over 8 cores, 65536 rows each):
 - Host folds BN into weights, collapses the expert head w3@wp -> wep, folds
   |wep| into w2's columns (so stage-3 reduction weights become exact +-1
   signs), and pre-transposes x into feature-major [64, BC] bf16 with a ones
   row at feature 59 (used to inject biases via accumulating matmuls).
 - Everything on-chip is bf16 matmul + f32 psum.  Per 512-row tile:
     stage1: 2 weights-stationary matmuls (experts 01 / 23), N=512
     gate hidden: 1 matmul into a 32-partition strip of a shared psum
     stage2: 2 block-diagonal K=128 matmuls, N=512
     stage3 preds / gate logits / biases: DATA-stationary matmuls - the
       activations (feature-major, in SBUF after relu) are the stationary
       operand and the tiny reduction weights stream, so each costs only
       N=2..8 moving columns.  Outputs land batch-major in one shared psum
       "tail" bank per 8192 rows: chunk cc -> cols [8cc:8cc+8] hold
       [p0 p1 p2 p3 l0 l1 l2 l3] for 128 rows.
 - Relu passes (psum->SBUF bf16 with per-partition bias) are spread across
   the Act, DVE and GPSIMD engines.
 - Tail per 8192 rows: spill bank to SBUF, exp(logits), w = p*exp, row
   reductions over the 4 experts, reciprocal, final product, DMA out.
"""

import numpy as np
import ml_dtypes

import concourse.bass as bass
import concourse.mybir as mybir
import concourse.tile as tile
from concourse import bacc
from concourse.bass_utils import run_bass_kernel_spmd

F32 = mybir.dt.float32
BF16 = mybir.dt.bfloat16
AF = mybir.ActivationFunctionType
ALU = mybir.AluOpType
AX = mybir.AxisListType

B, IN, E, H, EMB, GH = 524288, 59, 4, 64, 32, 32
EPS = 1e-5
NCORES = 8
BC = B // NCORES            # 65536 rows per core
NR = 8                      # rounds per core
RS = BC // NR               # 8192 rows per round
NT = RS // 512              # 16 tiles of 512 per round

# wb (bf16) column layout
W1A0, W1B0, GW10, W2A0, W2B0 = 0, 128, 256, 288, 416
SGA0, SGB0, GW2R0, BEP0 = 544, 546, 548, 552
WB_W = 560
# wf (f32) column layout: c1a c1b c2a c2b gb1t
WF_W = 8

_CACHE = {}

# relu engine assignment: per 16-tile round there are 68 psum->SBUF relu
# passes (64 tile + 4 gate).  GPSIMD cannot touch PSUM (BIR verifier), so
# they split across Act/DVE; Act is slightly faster per pass but also runs
# the exp, DVE runs the reductions/reciprocal.
def _relu_engines(total=36, quota=None):
    quota = quota or {"act": 6, "dve": 30}
    order = []
    frac = {k: 0.0 for k in quota}
    for _ in range(total):
        for k in frac:
            frac[k] += quota[k] / total
        pick = max(frac, key=lambda k: frac[k])
        frac[pick] -= 1.0
        order.append(pick)
    return order

RELU_ENG = _relu_engines()


def _build():
    nc = bacc.Bacc(trn_type="TRN2")
    x_d = nc.dram_tensor("x", (64, BC), BF16, kind="ExternalInput")
    wb_d = nc.dram_tensor("wb", (128, WB_W), BF16, kind="ExternalInput")
    wf_d = nc.dram_tensor("wf", (128, WF_W), F32, kind="ExternalInput")
    out_d = nc.dram_tensor("out", (NR, 128, 64), F32, kind="ExternalOutput")

    with tile.TileContext(nc) as tc:
        with (
            tc.tile_pool(name="consts", bufs=1) as consts,
            tc.tile_pool(name="xp", bufs=2) as xp,
            tc.tile_pool(name="h1p", bufs=2) as h1p,
            tc.tile_pool(name="h2p", bufs=2) as h2p,
            tc.tile_pool(name="g1p", bufs=2) as g1p,
            tc.tile_pool(name="tp", bufs=2) as tp,
            tc.tile_pool(name="p1", bufs=2, space="PSUM") as p1p,
            tc.tile_pool(name="p2a", bufs=1, space="PSUM") as p2ap,
            tc.tile_pool(name="p2b", bufs=1, space="PSUM") as p2bp,
            tc.tile_pool(name="pga", bufs=1, space="PSUM") as pgap,
            tc.tile_pool(name="ptl", bufs=1, space="PSUM") as ptlp,
        ):
            wb = consts.tile([128, WB_W], BF16)
            nc.sync.dma_start(out=wb, in_=wb_d[:, :])
            wf = consts.tile([128, WF_W], F32)
            nc.sync.dma_start(out=wf, in_=wf_d[:, :])

            w1a = wb[0:64, W1A0:W1A0 + 128]
            w1b = wb[0:64, W1B0:W1B0 + 128]
            gw1 = wb[0:64, GW10:GW10 + 32]
            w2a = wb[:, W2A0:W2A0 + 128]
            w2b = wb[:, W2B0:W2B0 + 128]
            sga = wb[:, SGA0:SGA0 + 2]
            sgb = wb[:, SGB0:SGB0 + 2]
            gw2r = wb[:, GW2R0:GW2R0 + 4]
            bep8 = wb[0:64, BEP0:BEP0 + 8]
            c2a = wf[:, 2:3]
            c2b = wf[:, 3:4]

            def relu(eng, out_sb, in_ps, bias_ap):
                if eng == "act":
                    nc.scalar.activation(
                        out_sb, in_ps, AF.Relu,
                        bias=bias_ap if bias_ap is not None else 0.0)
                elif bias_ap is None:
                    nc.vector.tensor_scalar(
                        out_sb, in_ps, 0.0, None, ALU.max)
                else:
                    nc.vector.tensor_scalar(
                        out_sb, in_ps, bias_ap, 0.0, ALU.add, ALU.max)

            for r in range(NR):
                x_sb = xp.tile([64, RS], BF16, tag="x")
                for ch in range(2):
                    cw = RS // 2
                    nc.sync.dma_start(
                        out=x_sb[:, ch * cw:(ch + 1) * cw],
                        in_=x_d[:, r * RS + ch * cw: r * RS + (ch + 1) * cw])

                tail = ptlp.tile([128, 512], F32, tag="tail")
                ri = 0  # relu slot index within round

                for g in range(4):
                    ga = pgap.tile([128, 512], F32, tag="ga")
                    g1r = g1p.tile([128, 512], BF16, tag="g1r")
                    h2s = []
                    for i in range(4):
                        t = 4 * g + i
                        c0 = 512 * t
                        xs = x_sb[:, c0:c0 + 512]

                        p1 = p1p.tile([128, 1024], F32, tag="p1")
                        nc.tensor.matmul(
                            out=p1[:, 0:512], lhsT=w1a, rhs=xs,
                            start=True, stop=True, skip_group_check=True)
                        nc.tensor.matmul(
                            out=p1[:, 512:1024], lhsT=w1b, rhs=xs,
                            start=True, stop=True, skip_group_check=True)
                        # gate hidden strip for this tile
                        nc.tensor.matmul(
                            out=ga[32 * i:32 * i + 32, :], lhsT=gw1, rhs=xs,
                            start=True, stop=True, skip_group_check=True,
                            tile_position=(0, 32 * i))

                        h1r = h1p.tile([128, 1024], BF16, tag="h1r")
                        nc.scalar.activation(h1r, p1, AF.Relu)

                        p2a = p2ap.tile([128, 512], F32, tag="p2a")
                        nc.tensor.matmul(
                            out=p2a, lhsT=w2a, rhs=h1r[:, 0:512],
                            start=True, stop=True)
                        p2b = p2bp.tile([128, 512], F32, tag="p2b")
                        nc.tensor.matmul(
                            out=p2b, lhsT=w2b, rhs=h1r[:, 512:1024],
                            start=True, stop=True)

                        h2ra = h2p.tile([128, 512], BF16, tag="h2ra")
                        relu(RELU_ENG[ri], h2ra, p2a, c2a); ri += 1
                        h2rb = h2p.tile([128, 512], BF16, tag="h2rb")
                        relu(RELU_ENG[ri], h2rb, p2b, c2b); ri += 1
                        h2s.append((t, c0, h2ra, h2rb))

                        for q in range(4):
                            col = 8 * (4 * t + q)
                            # bias row first: writes all 8 cols fresh with
                            # (bep, gb2) via the x ones-row; preds/logits
                            # then accumulate on top.
                            nc.tensor.matmul(
                                out=tail[:, col:col + 8],
                                lhsT=x_sb[:, c0 + 128 * q:c0 + 128 * q + 128],
                                rhs=bep8,
                                start=(t == 0 and q == 0), stop=False,
                                skip_group_check=True)
                            nc.tensor.matmul(
                                out=tail[:, col:col + 2],
                                lhsT=h2ra[:, 128 * q:128 * q + 128],
                                rhs=sga,
                                start=False, stop=False,
                                skip_group_check=True)
                            nc.tensor.matmul(
                                out=tail[:, col + 2:col + 4],
                                lhsT=h2rb[:, 128 * q:128 * q + 128],
                                rhs=sgb,
                                start=False, stop=False,
                                skip_group_check=True)

                    relu(RELU_ENG[ri], g1r, ga, None); ri += 1

                    for (t, c0, _, _) in h2s:
                        i = t - 4 * g
                        for q in range(4):
                            col = 8 * (4 * t + q) + 4
                            nc.tensor.matmul(
                                out=tail[:, col:col + 4],
                                lhsT=g1r[32 * i:32 * i + 32,
                                         128 * q:128 * q + 128],
                                rhs=gw2r[32 * i:32 * i + 32, :],
                                start=False, stop=(g == 3 and q == 3
                                                   and t == 4 * g + 3),
                                skip_group_check=True,
                                tile_position=(32 * i, 0))

                # ---- round tail: spill and combine (batch-major)
                tv = tail.rearrange("p (c k) -> p c k", k=8)
                expl = tp.tile([128, 256], F32, tag="expl")
                nc.scalar.activation(
                    expl.rearrange("p (c k) -> p c k", k=4),
                    tv[:, :, 4:8], AF.Exp)
                w_sb = tp.tile([128, 256], F32, tag="w")
                nc.vector.tensor_mul(
                    w_sb.rearrange("p (c k) -> p c k", k=4),
                    tv[:, :, 0:4],
                    expl.rearrange("p (c k) -> p c k", k=4))
                num = tp.tile([128, 64], F32, tag="num")
                nc.vector.tensor_reduce(
                    num, w_sb.rearrange("p (c k) -> p c k", k=4),
                    AX.X, ALU.add)
                den = tp.tile([128, 64], F32, tag="den")
                nc.vector.tensor_reduce(
                    den, expl.rearrange("p (c k) -> p c k", k=4),
                    AX.X, ALU.add)
                rec = tp.tile([128, 64], F32, tag="rec")
                nc.vector.reciprocal(rec, den)
                o_sb = tp.tile([128, 64], F32, tag="o")
                nc.gpsimd.tensor_mul(o_sb, num, rec)
                nc.sync.dma_start(out=out_d[r], in_=o_sb)

    if not nc.is_finalized():
        nc.finalize()
    return nc


def _pack_host(w1, b1, bn1_g, bn1_b, bn1_m, bn1_v, w2, b2, bn2_g, bn2_b,
               bn2_m, bn2_v, w3, b3, wp, bp, gw1, gb1, gw2, gb2):
    f = np.float32
    s1 = (bn1_g / np.sqrt(bn1_v + EPS)).astype(f)
    w1e = (w1 * s1[:, None, :]).astype(f)                       # (E,IN,H)
    c1 = ((b1 - bn1_m) * s1 + bn1_b).astype(f)                  # (E,H)
    s2 = (bn2_g / np.sqrt(bn2_v + EPS)).astype(f)
    w2e = (w2 * s2[:, None, :]).astype(f)                       # (E,H,H)
    c2 = ((b2 - bn2_m) * s2 + bn2_b).astype(f)                  # (E,H)
    wep = np.einsum("ehm,em->eh", w3, wp).astype(f)             # (E,H)
    bep = (np.einsum("em,em->e", b3, wp) + bp).astype(f)        # (E,)

    aw = np.abs(wep)                                            # (E,H)
    sg = np.sign(wep).astype(f)
    w2f = w2e * aw[:, None, :]                                  # cols scaled
    c2f = c2 * aw

    wb = np.zeros((128, WB_W), f)
    wb[0:IN, W1A0:W1A0 + 64] = w1e[0]
    wb[0:IN, W1A0 + 64:W1A0 + 128] = w1e[1]
    wb[0:IN, W1B0:W1B0 + 64] = w1e[2]
    wb[0:IN, W1B0 + 64:W1B0 + 128] = w1e[3]
    wb[IN, W1A0:W1A0 + 128] = np.concatenate([c1[0], c1[1]])
    wb[IN, W1B0:W1B0 + 128] = np.concatenate([c1[2], c1[3]])
    wb[0:IN, GW10:GW10 + 32] = gw1
    wb[IN, GW10:GW10 + 32] = gb1
    wb[0:64, W2A0:W2A0 + 64] = w2f[0]
    wb[64:128, W2A0 + 64:W2A0 + 128] = w2f[1]
    wb[0:64, W2B0:W2B0 + 64] = w2f[2]
    wb[64:128, W2B0 + 64:W2B0 + 128] = w2f[3]
    wb[0:64, SGA0] = sg[0]
    wb[64:128, SGA0 + 1] = sg[1]
    wb[0:64, SGB0] = sg[2]
    wb[64:128, SGB0 + 1] = sg[3]
    for gi in range(4):
        wb[32 * gi:32 * gi + 32, GW2R0:GW2R0 + 4] = gw2
    wb[IN, BEP0:BEP0 + 4] = bep
    wb[IN, BEP0 + 4:BEP0 + 8] = gb2

    wf = np.zeros((128, WF_W), f)
    wf[:, 2] = np.concatenate([c2f[0], c2f[1]])
    wf[:, 3] = np.concatenate([c2f[2], c2f[3]])
    return dict(wb=wb.astype(ml_dtypes.bfloat16), wf=wf)


def _x_core(xc):
    """(BC, 59) f32 -> (64, BC) bf16 feature-major with ones row at 59."""
    xt = np.zeros((64, BC), np.float32)
    xt[:IN] = xc.T
    xt[IN] = 1.0
    return np.ascontiguousarray(xt).astype(ml_dtypes.bfloat16)


def _unpack_out(o):
    """(NR, 128, 64) -> (BC,): row = 8192 r + 512 t + 128 q + b, col=4t+q."""
    o = np.asarray(o, np.float32).reshape(NR, 128, NT, 4)
    return np.ascontiguousarray(o.transpose(0, 2, 3, 1)).reshape(BC)


def _sim_inputs(x_full, packed):
    m = {"x": _x_core(np.asarray(x_full, np.float32)[:BC])}
    m.update(packed)
    return m


def kernel(**inputs):
    x = np.asarray(inputs["x"], dtype=np.float32)
    wk = {k: np.asarray(v, dtype=np.float32) for k, v in inputs.items()
          if k != "x"}
    packed = _pack_host(**wk)

    if "nc" not in _CACHE:
        _CACHE["nc"] = _build()
    nc = _CACHE["nc"]

    in_maps = []
    for c in range(NCORES):
        m = {"x": _x_core(x[c * BC:(c + 1) * BC])}
        m.update(packed)
        in_maps.append(m)

    res = run_bass_kernel_spmd(nc, in_maps, core_ids=list(range(NCORES)))
    _CACHE["last"] = res
    outs = [_unpack_out(r["out"]) for r in res.results]
    return np.concatenate(outs).reshape(B, 1).astype(np.float32)
